# revision 1
# baseline (speedup 1.0000x reference)
"""L1-loss kernel for Trainium2: mean over rows of sum(|out - target|).

Data-parallel over 8 NeuronCores: each core streams its row-shard of
`out` and `target` from HBM and produces per-partition partial sums of
|out - target|; the host sums the partials and divides by the global
row count.

Per core the shard is repacked host-side into [128, 2*FREE] tiles whose
partition rows hold the `out` chunk followed by the `target` chunk. One
DMA then feeds both operands of the subtract, which halves the DMA
count and keeps each compute instruction to a single upstream
semaphore. Per tile: DVE subtract into a scratch tile, then ACT Abs
with free-dim accumulation into an accumulator column — the two compute
engines each make one pass, fully hidden under the ~360 GB/s DMA stream
that bounds this memory-roofline workload.

Tail trimming: the last tile is packed as TAIL_CHUNKS quarter-tiles so
its compute pipelines with its loads, and the accumulator columns for
the first NT-1 tiles are flushed to DRAM early — only the tail columns
remain on the critical path after the final load.
"""

from contextlib import ExitStack

import numpy as np

import concourse.bass as bass
import concourse.bacc as bacc
import concourse.tile as tile
from concourse import mybir
from concourse.bass_utils import run_bass_kernel_spmd

N_VEH = 8388608
N_FEAT = 8
N_CORES = 8
ROWS_PER_CORE = N_VEH // N_CORES            # 1048576
ELEMS_PER_CORE = ROWS_PER_CORE * N_FEAT     # 8388608
P = 128
FREE = 2048
NT = ELEMS_PER_CORE // (P * FREE)           # 32 tiles; fused tile = [128, 4096] f32 (2 MiB)
TAIL_CHUNKS = 4                             # last tile split for tail overlap
H = FREE // TAIL_CHUNKS                     # tail chunk free size
NCOL = NT - 1 + TAIL_CHUNKS                 # NT-1 full columns + tail columns


def _build_nc() -> bass.Bass:
    # Bacc (not raw Bass): its compile() pass allocates registers and splits
    # multi-sem waits into EventSemaphore instructions — TRN2 instructions
    # fit only one wait. The PJRT exec path serializes the module as-is, so
    # finalize() must be called here.
    nc = bacc.Bacc()
    ot_ext = nc.declare_dram_parameter(
        "ot", [NT - 1, P, 2 * FREE], mybir.dt.float32, isOutput=False
    )
    ott_ext = nc.declare_dram_parameter(
        "ott", [TAIL_CHUNKS, P, 2 * H], mybir.dt.float32, isOutput=False
    )
    partials = nc.declare_dram_parameter(
        "partials", [P, NCOL], mybir.dt.float32, isOutput=True
    )

    with tile.TileContext(nc) as tc, ExitStack() as ctx:
        x_pool = ctx.enter_context(tc.tile_pool(name="x", bufs=6))
        d_pool = ctx.enter_context(tc.tile_pool(name="d", bufs=2))
        acc_pool = ctx.enter_context(tc.tile_pool(name="acc", bufs=1))
        acc = acc_pool.tile([P, NCOL], mybir.dt.float32)
        for i in range(NT - 1):
            x = x_pool.tile([P, 2 * FREE], mybir.dt.float32)
            nc.sync.dma_start(x[:], ot_ext[i])
            d = d_pool.tile([P, FREE], mybir.dt.float32)
            nc.vector.tensor_tensor(
                out=d[:], in0=x[:, :FREE], in1=x[:, FREE:],
                op=mybir.AluOpType.subtract,
            )
            nc.scalar.activation(
                out=d[:], in_=d[:],
                func=mybir.ActivationFunctionType.Abs,
                accum_out=acc[:, i : i + 1],
            )
        xs = []
        for k in range(TAIL_CHUNKS):
            xk = x_pool.tile([P, 2 * H], mybir.dt.float32, tag="xtail")
            nc.sync.dma_start(xk[:], ott_ext[k])
            xs.append(xk)
        nc.sync.dma_start(partials[:, : NT - 1], acc[:, : NT - 1])
        for k in range(TAIL_CHUNKS):
            dk = d_pool.tile([P, H], mybir.dt.float32, tag="dtail")
            nc.vector.tensor_tensor(
                out=dk[:], in0=xs[k][:, :H], in1=xs[k][:, H:],
                op=mybir.AluOpType.subtract,
            )
            nc.scalar.activation(
                out=dk[:], in_=dk[:],
                func=mybir.ActivationFunctionType.Abs,
                accum_out=acc[:, NT - 1 + k : NT + k],
            )
        nc.sync.dma_start(partials[:, NT - 1 :], acc[:, NT - 1 :])
    nc.finalize()
    return nc


def _pack(out: np.ndarray, target: np.ndarray) -> list[dict[str, np.ndarray]]:
    """Interleave out/target per partition row; last tile as two half-tiles."""
    in_maps = []
    for c in range(N_CORES):
        sl = slice(c * ROWS_PER_CORE, (c + 1) * ROWS_PER_CORE)
        o = out[sl].reshape(NT, P, FREE)
        t = target[sl].reshape(NT, P, FREE)
        ot = np.empty((NT - 1, P, 2 * FREE), dtype=np.float32)
        ot[:, :, :FREE] = o[: NT - 1]
        ot[:, :, FREE:] = t[: NT - 1]
        ott = np.empty((TAIL_CHUNKS, P, 2 * H), dtype=np.float32)
        for k in range(TAIL_CHUNKS):
            ott[k, :, :H] = o[NT - 1, :, k * H : (k + 1) * H]
            ott[k, :, H:] = t[NT - 1, :, k * H : (k + 1) * H]
        in_maps.append({"ot": ot, "ott": ott})
    return in_maps


def _run(nc: bass.Bass, out: np.ndarray, target: np.ndarray, **kwargs):
    return run_bass_kernel_spmd(nc, _pack(out, target), list(range(N_CORES)), **kwargs)


def kernel(out: np.ndarray, target: np.ndarray, x: np.ndarray | None = None) -> np.ndarray:
    out = np.ascontiguousarray(np.asarray(out, dtype=np.float32))
    target = np.ascontiguousarray(np.asarray(target, dtype=np.float32))
    res = _run(_build_nc(), out, target)
    total = sum(r["partials"].astype(np.float64).sum() for r in res.results)
    return np.asarray(total / N_VEH, dtype=np.float32)



# revision 2
# speedup vs baseline: 4.1408x; 4.1408x over previous
"""L1-loss kernel for Trainium2: mean over rows of sum(|out - target|).

Data-parallel over 8 NeuronCores. Host-side, each core's row-shard of
`out`/`target` is cast to fp16 and repacked into fused [128, 2*F] tiles
(a-chunk || b-chunk per partition row), so one DMA feeds both operands.

On-device, the work is spread across all four issue-capable engines:
  - loads are striped over the three DMA-capable queues (SP / Activation
    / Pool), which stream concurrently;
  - the subtract runs on DVE (tensor_tensor, 2x fp16 mode) for most
    tiles and on Pool (tensor_tensor) for the rest;
  - |d| + free-axis sum-accumulate runs on DVE's tensor_scalar
    (abs_max with op1=add reduction, 4x fp16 mode) into per-tile
    accumulator columns.

The host sums the per-partition partial columns in float64 and divides
by the global row count. fp16 rounding of the inputs/difference adds
~1e-6 relative error to the final mean, far inside the 2e-2 gate.
"""

from contextlib import ExitStack

import numpy as np

import concourse.bass as bass
import concourse.bacc as bacc
import concourse.tile as tile
from concourse import mybir
from concourse.bass_utils import run_bass_kernel_spmd

N_VEH = 8388608
N_FEAT = 8
N_CORES = 8
ROWS_PER_CORE = N_VEH // N_CORES            # 1048576
ELEMS_PER_CORE = ROWS_PER_CORE * N_FEAT     # 8388608 per tensor
P = 128
F = 2048                                    # output elems per partition per tile
NT = ELEMS_PER_CORE // (P * F)              # 32 tiles; fused tile [128, 4096] fp16

# --- schedule: per-tile queue + subtract-engine assignment ---------------
# Queues: 'S' = SP (sync), 'A' = Activation (scalar), 'P' = Pool (gpsimd).
# Loads per queue (nS + nA + nP = NT) and Pool-subtract count balance the
# four engines: SP/ACT ~ nS*3158ns, Pool ~ nP*3158 + subP*1707,
# DVE ~ (NT-subP)*1127 + NT*552.
N_S, N_A, N_P = 13, 13, 6
SUB_P = 12                                  # tiles whose subtract runs on Pool


def _make_schedule():
    """Interleave queue assignments so tile order ~ load completion order."""
    order = []
    cnt = {"S": N_S, "A": N_A, "P": N_P}
    err = {"S": 0.0, "A": 0.0, "P": 0.0}
    for _ in range(NT):
        for q in cnt:
            err[q] += cnt[q] / NT
        q = max(err, key=lambda k: err[k])
        err[q] -= 1.0
        order.append(q)
    # Pool subtracts: attach to non-Pool-loaded tiles, spread evenly,
    # avoiding the first few (warmup) and last few (tail) tiles.
    non_pool = [i for i, q in enumerate(order) if q != "P"]
    picks = set()
    if SUB_P:
        step = len(non_pool) / SUB_P
        picks = {non_pool[min(int((k + 0.5) * step), len(non_pool) - 1)]
                 for k in range(SUB_P)}
    sub = ["P" if i in picks else "V" for i in range(NT)]
    return order, sub


QUEUE, SUB = _make_schedule()


def _build_nc() -> bass.Bass:
    nc = bacc.Bacc()
    ot = nc.declare_dram_parameter("ot", [NT, P, 2 * F], mybir.dt.float16,
                                   isOutput=False)
    partials = nc.declare_dram_parameter("partials", [P, NT], mybir.dt.float32,
                                         isOutput=True)
    eng = {"S": None, "A": None, "P": None}

    with tile.TileContext(nc) as tc, ExitStack() as ctx:
        eng = {"S": nc.sync, "A": nc.scalar, "P": nc.gpsimd}
        x_pool = ctx.enter_context(tc.tile_pool(name="x", bufs=12))
        d_pool = ctx.enter_context(tc.tile_pool(name="d", bufs=6))
        acc_pool = ctx.enter_context(tc.tile_pool(name="acc", bufs=1))
        acc = acc_pool.tile([P, NT], mybir.dt.float32)
        for i in range(NT):
            x = x_pool.tile([P, 2 * F], mybir.dt.float16)
            eng[QUEUE[i]].dma_start(x[:], ot[i])
            d = d_pool.tile([P, F], mybir.dt.float16)
            sub_eng = nc.gpsimd if SUB[i] == "P" else nc.vector
            sub_eng.tensor_tensor(out=d[:], in0=x[:, :F], in1=x[:, F:],
                                  op=mybir.AluOpType.subtract)
            nc.vector.tensor_scalar(out=d[:], in0=d[:], scalar1=0.0,
                                    scalar2=None, op0=mybir.AluOpType.abs_max,
                                    op1=mybir.AluOpType.add,
                                    accum_out=acc[:, i:i + 1])
        nc.sync.dma_start(partials[:], acc[:])
    nc.finalize()
    return nc


def _pack(out: np.ndarray, target: np.ndarray) -> list[dict[str, np.ndarray]]:
    """Cast to fp16 and fuse out/target chunks per partition row."""
    o16 = out.astype(np.float16).reshape(N_CORES, NT, P, F)
    t16 = target.astype(np.float16).reshape(N_CORES, NT, P, F)
    in_maps = []
    for c in range(N_CORES):
        ot = np.empty((NT, P, 2 * F), dtype=np.float16)
        ot[:, :, :F] = o16[c]
        ot[:, :, F:] = t16[c]
        in_maps.append({"ot": ot})
    return in_maps


def _run(nc: bass.Bass, out: np.ndarray, target: np.ndarray, **kwargs):
    return run_bass_kernel_spmd(nc, _pack(out, target), list(range(N_CORES)),
                                **kwargs)


def kernel(out: np.ndarray, target: np.ndarray,
           x: np.ndarray | None = None) -> np.ndarray:
    out = np.ascontiguousarray(np.asarray(out, dtype=np.float32))
    target = np.ascontiguousarray(np.asarray(target, dtype=np.float32))
    res = _run(_build_nc(), out, target)
    total = sum(r["partials"].astype(np.float64).sum() for r in res.results)
    return np.asarray(total / N_VEH, dtype=np.float32)
